# revision 25
# baseline (speedup 1.0000x reference)
"""Fused causal MHA kernel for TRN2, one core = (batch b, head-group g of 8 heads).

Layouts (per core):
  xt   [1024, N]     X[b]^T                 (k on partitions)
  wq/wk/wv [1024, 512] column shard         (k on partitions)
  wo   [512, 1024]   row shard              (dv on partitions)
  maskt [128, 4*512] transposed relative causal mask tiles r=0..3:
        maskt[j, r*512+i] = mask[i, 128*r+j]  (0 / -1e9)
  outt [1024, N]     partial (X attn Wo_g)^T ; host sums the two
        head-group partials per batch and transposes.

On-chip:
  qt/kt per head-pair hp: [128, N]; partitions = (h0 d0-63, h1 d0-63).
  v per seq m-block: [128, 512]; seq on partitions, 8 heads * 64 on free.
  S^T per (hp, c, jb): psum [128, 1024] = h0|h1; j on partitions, i on free.
  PV col-packed: psumO[0:64] = h0 O^T, [64:128] = h1 O^T.
  denom via ones-lhsT matmul into psumD with the same packing, so the
  reciprocal+scale runs lane-aligned on DVE with no partition broadcast.
"""

import numpy as np
import concourse.bass as bass
import concourse.tile as tile
from concourse import bacc, mybir

F32R = mybir.dt.float32r
F32 = mybir.dt.float32
F16 = mybir.dt.float16
AF = mybir.ActivationFunctionType

P = 128
D = 1024
DH = 512  # head-group width: 8 heads * 64
DK = 64
KB = D // P  # 8 k-blocks
MOFF = [0, 128, 384, 768]  # mask prefix offsets per r
NHP = 4  # head-pairs per core


def build(N=2048, interleave=True):
    MB = N // P  # seq 128-blocks
    MC = N // 512  # seq 512-chunks
    nc = bacc.Bacc("TRN2", target_bir_lowering=False, debug=False)

    xt_d = nc.dram_tensor("xt", [D, N], F16, kind="ExternalInput")
    wq_d = nc.dram_tensor("wq", [D, DH], F16, kind="ExternalInput")
    wk_d = nc.dram_tensor("wk", [D, DH], F16, kind="ExternalInput")
    wv_d = nc.dram_tensor("wv", [D, DH], F16, kind="ExternalInput")
    wo_d = nc.dram_tensor("wo", [DH, D], F16, kind="ExternalInput")
    mask_d = nc.dram_tensor("maskt", [P, P], F32, kind="ExternalInput")
    ones_d = nc.dram_tensor("ones16", [P, DK], F16, kind="ExternalInput")
    out_d = nc.dram_tensor("outt", [D, N], F32, kind="ExternalOutput")

    with tile.TileContext(nc) as tc:
        with (
            tc.tile_pool(name="sb", bufs=1) as sb,
            tc.tile_pool(name="ps", bufs=1, space="PSUM") as ps,
        ):
            # ---- persistent tiles ----
            xt = [sb.tile([P, N], F16, tag="xt", bufs=8, name=f"xt{k}") for k in range(KB)]
            wv = [sb.tile([P, DH], F16, tag="sm512", bufs=14, name=f"wv{k}") for k in range(KB)]
            v = [sb.tile([P, 8 * 65], F16, tag="v", bufs=MB, name=f"v{m}") for m in range(MB)]
            ot = [sb.tile([P, N], F16, tag="ot", bufs=NHP, name=f"ot{t}") for t in range(NHP)]
            maskt = sb.tile([P, P], F32, tag="maskt", bufs=1)
            ones = sb.tile([P, DK], F16, tag="ones", bufs=1)

            for k in range(KB):
                nc.sync.dma_start(wv[k][:], wv_d.ap()[k * P:(k + 1) * P, :])
            for cc in range(0, N, 512):
                for k in range(KB):
                    nc.sync.dma_start(
                        xt[k][:, cc:cc + 512],
                        xt_d.ap()[k * P:(k + 1) * P, cc:cc + 512],
                    )
            nc.sync.dma_start(maskt[:], mask_d.ap())
            nc.sync.dma_start(ones[:], ones_d.ap())
            # HAM warm-up: ~3.5us of dummy matmuls at t=0 so the PE clock
            # is at 2.4GHz by the time real work (and its DMAs) arrive.
            warmw = sb.tile([P, 512], F16, tag="warmw", bufs=1, name="warmw")
            nc.gpsimd.memset(warmw[:], 0.0)
            psW = ps.tile([P, 512], F32, tag="proj", bufs=2, name="psW")
            for _ in range(22):
                nc.tensor.matmul(psW[:], warmw[:, 0:P], warmw[:], start=True, stop=True)
            warmc = sb.tile([P, 512], F16, tag="warmw", bufs=1, name="warmc")
            nc.vector.tensor_copy(warmc[:], psW[:])
            # warm the ACT exp table set during the DMA lead-in
            warm = sb.tile([P, DK], F16, tag="warm", bufs=1, name="warm")
            nc.scalar.activation(warm[:], ones[:], AF.Exp)
            wo_t = [
                sb.tile([P, D], F16, tag="wo", bufs=NHP, name=f"wo{dv}")
                for dv in range(NHP)
            ]

            # ---- deferred projection work (pumped between attention units) ----
            # entries are (deadline, fn); deadline is a (t, c) chunk key or
            # None. FIFO pump; force_drain emits everything due before a
            # chunk so reads never precede their producer in program order.
            deferred = []
            dve_deferred = []
            credit = [0.0]
            hold = [0]

            def pump(rate):
                if dve_deferred:
                    dve_deferred.pop(0)()
                credit[0] += rate
                while credit[0] >= 1.0 and len(deferred) > hold[0]:
                    deferred.pop(0)[1]()
                    credit[0] -= 1.0
                if len(deferred) <= hold[0]:
                    credit[0] = 0.0

            def force_drain(upto):
                while deferred and deferred[0][0] is not None and deferred[0][0] <= upto:
                    deferred.pop(0)[1]()

            def v_proj(m):
                psV = ps.tile([P, 512], F32, tag="proj", bufs=2, name="psV")
                for k in range(KB):
                    nc.tensor.matmul(
                        psV[:],
                        xt[k][:, m * P:(m + 1) * P],
                        wv[k][:],
                        start=(k == 0),
                        stop=(k == KB - 1),
                    )
                v3 = v[m][:].rearrange("p (h x) -> p h x", x=65)
                nc.vector.tensor_copy(
                    v3[:, :, 0:64], psV[:].rearrange("p (h x) -> p h x", x=64)
                )
                nc.vector.tensor_copy(v3[:, :, 64:65], ones[:, 0:8, None])

            def qk_proj_parts(hp, c, w_tiles, dst, scale):
                cell = {}

                def part(k0, k1, fin):
                    if k0 == 0:
                        cell["ps"] = ps.tile(
                            [P, 512], F32, tag="proj", bufs=2, name="psQ"
                        )
                    psQ = cell["ps"]
                    for k in range(k0, k1):
                        nc.tensor.matmul(
                            psQ[:],
                            w_tiles[k][:],
                            xt[k][:, c * 512:(c + 1) * 512],
                            start=(k == 0),
                            stop=(k == KB - 1),
                        )
                    if fin:
                        # scale (1/sqrt(DK)) is folded into the exp ACT's
                        # free affine, so Q and K both finalize as a copy
                        nc.vector.tensor_copy(
                            dst[:, c * 512:(c + 1) * 512], psQ[:]
                        )

                return [
                    lambda: part(0, 4, False),
                    lambda: part(4, KB, True),
                ]

            qt = {}
            kt = {}

            def qk_work(hp):
                qt[hp] = sb.tile([P, N], F16, tag="qt", bufs=3, name=f"qt{hp}")
                kt[hp] = sb.tile([P, N], F16, tag="kt", bufs=3, name=f"kt{hp}")
                wqt = [sb.tile([P, P], F16, tag="wq", bufs=16, name=f"wq{hp}_{k}") for k in range(KB)]
                wkt = [sb.tile([P, P], F16, tag="wk", bufs=16, name=f"wk{hp}_{k}") for k in range(KB)]
                for k in range(KB):
                    nc.sync.dma_start(
                        wqt[k][:], wq_d.ap()[k * P:(k + 1) * P, hp * P:(hp + 1) * P]
                    )
                    nc.sync.dma_start(
                        wkt[k][:], wk_d.ap()[k * P:(k + 1) * P, hp * P:(hp + 1) * P]
                    )
                out = []
                for c in range(MC):
                    for fn in qk_proj_parts(hp, c, wqt, qt[hp], None):
                        out.append(((hp, c), fn))
                    for fn in qk_proj_parts(hp, c, wkt, kt[hp], None):
                        out.append(((hp, c), fn))
                return out

            def attn_chunk(hp, c, pump_rate=0.5, norm_q=None, prepend_norm=False):
                jb_max = min(MB, 4 * c + 4)
                psOa = [
                    ps.tile([65, 512], F32, tag="psO", bufs=2, name="psO0"),
                    ps.tile([65, 512], F32, tag="psO", bufs=2, name="psO1"),
                ]
                pts = {}

                def stage_s(jb):
                    psS = ps.tile([P, 1024], F32, tag="psS", bufs=2, name="psS")
                    r = jb - 4 * c
                    pre = P * r if r > 0 else 0
                    for h2 in range(2):
                        nc.tensor.matmul(
                            psS[:, h2 * 512 + pre:(h2 + 1) * 512],
                            kt[hp][h2 * DK:(h2 + 1) * DK, jb * P:(jb + 1) * P],
                            qt[hp][h2 * DK:(h2 + 1) * DK, c * 512 + pre:(c + 1) * 512],
                            start=True,
                            stop=True,
                            tile_position=(h2 * DK, 0),
                        )
                    if r >= 0:
                        # only the 128-wide triangle needs the additive mask;
                        # columns below the prefix are fully masked and are
                        # zeroed in pt after the exp instead
                        for h2 in range(2):
                            nc.vector.tensor_add(
                                psS[:, h2 * 512 + pre:h2 * 512 + pre + P],
                                psS[:, h2 * 512 + pre:h2 * 512 + pre + P],
                                maskt[:],
                            )
                    pt = sb.tile([P, 1024], F16, tag="pt", bufs=4, name="pt")
                    # 1/sqrt(DK) rides the ACT's free affine (scale); the
                    # masked prefix is never read by PV so it's left stale.
                    if pre:
                        # one strided ACT over both heads' valid slices
                        psS3 = psS[:].rearrange("p (h x) -> p h x", h=2)
                        pt3 = pt[:].rearrange("p (h x) -> p h x", h=2)
                        nc.scalar.activation(
                            pt3[:, :, pre:512], psS3[:, :, pre:512], AF.Exp,
                            scale=0.125,
                        )
                    else:
                        nc.scalar.activation(pt[:], psS[:], AF.Exp, scale=0.125)
                    pts[jb] = pt

                def stage_pv(jb):
                    pt = pts.pop(jb)
                    first, last = (jb == 0), (jb == jb_max - 1)
                    r = jb - 4 * c
                    pre = P * r if (r > 0 and not first) else 0
                    for h2 in range(2):
                        h = 2 * hp + h2
                        nc.tensor.matmul(
                            psOa[h2][0:65, pre:512],
                            v[jb][:, h * 65:(h + 1) * 65],
                            pt[:, h2 * 512 + pre:(h2 + 1) * 512],
                            start=first,
                            stop=last,
                            skip_group_check=True,
                        )
                    pump(pump_rate)

                for jb in range(jb_max):
                    stage_s(jb)
                    if jb >= 2:
                        stage_pv(jb - 2)
                stage_pv(jb_max - 2)
                stage_pv(jb_max - 1)

                cpO = [
                    sb.tile([65, 512], F32, tag="sm512", bufs=14, name=f"cpO{h2}")
                    for h2 in range(2)
                ]
                nc.vector.tensor_copy(cpO[0][0:65, :], psOa[0][0:65, :])
                nc.vector.tensor_copy(cpO[1][0:65, :], psOa[1][0:65, :])
                rbc = [
                    sb.tile([64, 512], F32, tag="sm512", bufs=14, name=f"rbc{h2}")
                    for h2 in range(2)
                ]
                tmp1 = sb.tile([64, 512], F16, tag="sm512", bufs=14, name="tmp1")

                nr = sb.tile([1, 1024], F32, tag="nr", bufs=4, name="nr")
                nr2 = sb.tile([1, 1024], F32, tag="nr", bufs=4, name="nr2")

                def norm_piece(stage):
                    if stage == 0:
                        # move denominator rows (lane 64) to lane 0
                        nc.sync.dma_start(nr[0:1, 0:512], cpO[0][64:65, :])
                        nc.sync.dma_start(nr[0:1, 512:1024], cpO[1][64:65, :])
                    elif stage == 1:
                        nc.vector.reciprocal_approx_fast(nr2[0:1, :], nr[0:1, :])
                    elif stage == 2:
                        nc.gpsimd.partition_broadcast(
                            rbc[0][0:64, :], nr2[0:1, 0:512]
                        )
                        nc.gpsimd.partition_broadcast(
                            rbc[1][0:64, :], nr2[0:1, 512:1024]
                        )
                    elif stage == 3:
                        nc.vector.tensor_tensor(
                            ot[hp][0:64, c * 512:(c + 1) * 512],
                            cpO[0][0:64, :],
                            rbc[0][0:64, :],
                            mybir.AluOpType.mult,
                        )
                    elif stage == 4:
                        nc.vector.tensor_tensor(
                            tmp1[0:64, :],
                            cpO[1][0:64, :],
                            rbc[1][0:64, :],
                            mybir.AluOpType.mult,
                        )
                        nc.sync.dma_start(
                            ot[hp][64:128, c * 512:(c + 1) * 512], tmp1[0:64, :]
                        )

                if interleave:
                    if norm_q is None:
                        for st in range(5):
                            dve_deferred.append(lambda st=st: norm_piece(st))
                    elif prepend_norm:
                        stages = [
                            (None, lambda st=st: norm_piece(st)) for st in range(5)
                        ]
                        norm_q[0:0] = stages[0:2]
                        pos = min(8, len(norm_q))
                        norm_q[pos:pos] = stages[2:5]
                    else:
                        for st in range(5):
                            norm_q.append((None, lambda st=st: norm_piece(st)))
                else:
                    for st in range(5):
                        norm_piece(st)

            # ---- schedule ----

            def outproj_parts(do, c):
                cell = {}

                def part(v0, v1, fin):
                    if v0 == 0:
                        cell["ps"] = ps.tile(
                            [P, 512], F32, tag="proj", bufs=2, name="psF"
                        )
                    psF = cell["ps"]
                    for dv in range(v0, v1):
                        nc.tensor.matmul(
                            psF[:],
                            wo_t[dv][:, do * P:(do + 1) * P],
                            ot[dv][:, c * 512:(c + 1) * 512],
                            start=(dv == 0),
                            stop=(dv == NHP - 1),
                        )
                    if fin:
                        o_sb = sb.tile([P, 512], F32, tag="sm512", bufs=14, name="o_sb")
                        if c == MC - 1:
                            # tail chunk: ACT is done with exps, use it for
                            # the PSUM bounce so DVE never gates psF reuse
                            nc.scalar.copy(o_sb[:], psF[:])
                        else:
                            nc.vector.tensor_copy(o_sb[:], psF[:])
                        nc.sync.dma_start(
                            out_d.ap()[do * P:(do + 1) * P, c * 512:(c + 1) * 512],
                            o_sb[:],
                        )

                return [lambda: part(0, 2, False), lambda: part(2, NHP, True)]

            def units_in(chunks):
                return sum(min(MB, 4 * cc + 4) for cc in chunks)

            for dv in range(NHP):
                nc.sync.dma_start(wo_t[dv][:], wo_d.ap()[dv * P:(dv + 1) * P, :])
            qk0 = qk_work(0)
            for m in range(MB):
                v_proj(m)
            for _, fn in qk0:
                fn()

            for t in range(NHP):
                nxt = qk_work(t + 1) if t + 1 < NHP else []
                if interleave:
                    deferred.extend(nxt)
                last = t == NHP - 1
                for ci, c in enumerate(range(MC)):
                    rem = units_in(range(c, MC))
                    reserve = 6 if last else 0
                    hold[0] = reserve
                    rate = min(
                        3.0,
                        max(0.0, len(deferred) - reserve) / max(rem, 1)
                        + 0.05,
                    )
                    attn_chunk(t, c, pump_rate=rate,
                               norm_q=deferred if last else None,
                               prepend_norm=last and ci == MC - 1)
                    if last and interleave:
                        for do in range(D // P):
                            for fn in outproj_parts(do, c):
                                deferred.append((None, fn))
                if t < NHP - 1:
                    # interleave PE (deferred proj parts) with DVE/DMA (norm
                    # stages) so neither engine bubbles at the hp boundary
                    while dve_deferred or deferred:
                        if dve_deferred:
                            dve_deferred.pop(0)()
                        if deferred:
                            deferred.pop(0)[1]()
                if not interleave:
                    for _, fn in nxt:
                        fn()

            # ---- drain remaining deferred work ----
            while dve_deferred:
                dve_deferred.pop(0)()
            while deferred:
                deferred.pop(0)[1]()
            if not interleave:
                for c in range(MC):
                    for do in range(D // P):
                        for th in outproj_parts(do, c):
                            th()

    nc.compile()
    return nc


def make_core_inputs(X, mask, Wq, Wk, Wv, Wo):
    """Full inputs -> list of 8 per-core input maps (batch-major, head-group minor)."""
    B = X.shape[0]
    maskt = np.ascontiguousarray(mask[0:P, 0:P].T.astype(np.float32))
    in_maps = []
    for b in range(B):
        xt = np.ascontiguousarray(X[b].T.astype(np.float16))
        for g in range(2):
            sl = slice(g * DH, (g + 1) * DH)
            in_maps.append(
                {
                    "xt": xt,
                    "wq": np.ascontiguousarray(Wq[:, sl].astype(np.float16)),
                    "wk": np.ascontiguousarray(Wk[:, sl].astype(np.float16)),
                    "wv": np.ascontiguousarray(Wv[:, sl].astype(np.float16)),
                    "wo": np.ascontiguousarray(Wo[sl, :].astype(np.float16)),
                    "maskt": maskt,
                    "ones16": np.ones((P, DK), np.float16),
                }
            )
    return in_maps


def gather_output(results, B=4):
    N = results[0]["outt"].shape[1]
    out = np.empty((B, N, D), np.float32)
    for b in range(B):
        s = results[2 * b]["outt"] + results[2 * b + 1]["outt"]
        out[b] = s.T
    return out


# ---------------------------------------------------------------------------
# Self-contained harness entry: full inputs in, full output out.
# Shards across 8 NeuronCores: core = batch b (4) x head-group g (2 x 8 heads).
# Each core runs a fused flash-style causal MHA for its 8 heads; the host
# sums the two head-group partial outputs per batch (row-parallel W_O).
# ---------------------------------------------------------------------------
from concourse.bass_utils import run_bass_kernel_spmd

_NC_CACHE = {}


def _get_nc():
    if "nc" not in _NC_CACHE:
        _NC_CACHE["nc"] = build(N=2048, interleave=True)
    return _NC_CACHE["nc"]


def kernel(X, mask, Wq, Wk, Wv, Wo):
    X = np.asarray(X, dtype=np.float32)
    mask = np.asarray(mask, dtype=np.float32)
    Wq = np.asarray(Wq, dtype=np.float32)
    Wk = np.asarray(Wk, dtype=np.float32)
    Wv = np.asarray(Wv, dtype=np.float32)
    Wo = np.asarray(Wo, dtype=np.float32)
    in_maps = make_core_inputs(X, mask, Wq, Wk, Wv, Wo)
    nc = _get_nc()
    res = run_bass_kernel_spmd(nc, in_maps, list(range(8)))
    return gather_output(res.results, B=X.shape[0])

